# revision 28
# baseline (speedup 1.0000x reference)
"""Trainium2 Bass kernel for nn_EternalRecursion (GRUCell self-recursion, B=512, D=500).

Strategy
--------
Data-parallel over 8 NeuronCores: 64 batch rows per core, GRU weights replicated.

Math restructuring (host-side, exact):
  - After step 1 the reference feeds h_new as BOTH x and h of the GRU cell, so
    steps >= 2 use combined weights W_rz = (W_ih+W_hh)[0:1000] for the r/z gates,
    while the n-gate keeps W_ih_n / W_hh_n separate (r multiplies only the h-side).
  - Step 1 (x=state, h=0) uses W_ih with a zero block for the h-side n columns,
    which makes it the *same* device code path with different weights.
  - Biases are folded into the matmul via an extra contraction row of ones.
  - The break check "mean(h_k) > bc" latches the output at the first step k*
    whose global mean exceeds bc. The device free-runs L steps, records per-step
    per-partition sums (free side-output of the last fused DVE op), and the host
    computes the global means. If the break fires before the last step (it cannot
    for the harness inputs: |h|<1 and bc>=0.9 keeps means far below bc), the
    kernel is re-built with L=k* and re-run, which reproduces the latched output.

Device layout (per core, per step):
  - h is stored "packed": [128 partitions, 250 free] with partition 64*H+b
    holding h[b, 250*H + c]. All elementwise gate math runs on [128, 250] tiles.
  - Gate pre-activations are computed as 16 f32r matmuls with M=128 into two
    [128, 512] PSUM tiles: region 1 = [gr | gz] (sigmoids + the z-path overlap
    region 2's matmuls), region 2 = [gin | ghn]. Since f32r matmuls require
    dst partition base 0, both packed halves are produced by ONE matmul via a
    doubled contraction: the stationary holds h^T twice along K (gate-half G0
    channels with batch in array cols 0:64 + zeros, G1 channels in cols
    64:128), and the moving W rows carry the matching half's gate columns.
    8 K-tile groups x 2 regions of N=500.
  - The stationary h^T lives in one [126, 1024] SBUF tile; groups are ordered
    (D0,G0),(D1,G0),(D0,G1),(D1,G1),(D2,G0),(D3,G0),(D2,G1),(D3,G1) so the
    first four matmuls depend only on PE-transpose A of h_new, whose two
    PSUM->SBUF copies run in parallel on ACT and DVE. Ones rows in groups 1/3
    implement the bias fold; the complementary halves stay zero. The gate
    chain is split in two 125-column chunks so transpose A starts while chunk
    1 is still in the vector/scalar pipes.
"""

import os
import sys
import types
import numpy as np

D = 500
B = 512
NCORES = 8
BS = B // NCORES          # 64 batch rows per core
HALF = 250                # free columns of the packed layout
# K permutation: hT column-groups are [0:125 | 250:375 | 125:250 | 375:500]
PERM = np.concatenate([
    np.arange(0, 125), np.arange(250, 375),
    np.arange(125, 250), np.arange(375, 500),
])
# gate rows for the rz column blocks: [r 0:250 | z 0:250 | r 250:500 | z 250:500]
GATE_ROWS_RZ = np.concatenate([
    np.arange(0, 250), np.arange(500, 750),
    np.arange(250, 500), np.arange(750, 1000),
])


def _install_hook_module():
    """Provide antenv.axon_hooks (missing from the RO image) so NTFF tracing
    through bass_utils can work when requested. Harmless if anything fails."""
    if "antenv.axon_hooks" in sys.modules:
        return
    mod = types.ModuleType("antenv.axon_hooks")
    holder = [None]
    mod.set_axon_ntff_profile_hook = lambda h: holder.__setitem__(0, h)
    mod.get_axon_ntff_profile_hook = lambda: holder[0]
    sys.modules["antenv.axon_hooks"] = mod
    try:
        from trn_agent_boot.trn_boot import _ntff_profile_via_ctypes
        hook = _ntff_profile_via_ctypes("/opt/axon/libaxon_pjrt.so")
        mod.set_axon_ntff_profile_hook(hook)
    except Exception:
        pass


_install_hook_module()

import concourse.bass as bass  # noqa: E402
import concourse.mybir as mybir  # noqa: E402
import concourse.tile as tile  # noqa: E402
from concourse import bass_utils  # noqa: E402
from concourse.masks import make_identity  # noqa: E402
import bass_rust  # noqa: E402

F32 = mybir.dt.float32
F32R = mybir.dt.float32r
AF = mybir.ActivationFunctionType
ALU = mybir.AluOpType


def _split_overwide_waits(nc, maxw=1):
    """walrus here rejects >1 sync wait per instruction; spread extras over
    preceding NoOp carriers. Most multi-wait instructions get same-engine
    carriers (order-preserving); the kernel-end drain (many loose-end waits)
    gets carriers round-robined across all engines so they resolve in
    parallel before the final barrier instead of serially on one engine."""
    n_new = 0
    all_engines = (mybir.EngineType.SP, mybir.EngineType.Activation,
                   mybir.EngineType.PE, mybir.EngineType.DVE,
                   mybir.EngineType.Pool)
    for fn in nc.m.functions:
        for bb in fn.blocks:
            out = []
            for inst in bb.instructions:
                si = inst.sync_info
                if si is not None and si.on_wait and len(si.on_wait) > maxw:
                    waits = list(si.on_wait)
                    chunks = [waits[i:i + maxw] for i in range(0, len(waits), maxw)]
                    spread = len(chunks) > 4  # only the big end-of-kernel drain
                    for j, ch in enumerate(chunks[:-1]):
                        eng = all_engines[j % len(all_engines)] if spread                             else inst.engine
                        nd = mybir.InstNoOp(
                            name=f"I-swx{n_new}", engine=eng,
                            bass_nofuse=True,
                            sync_info=bass_rust.SyncInfo(on_wait=ch, on_update=[]))
                        n_new += 1
                        nc.register_instruction(nd, overwrite=True)
                        out.append(nd)
                    inst.sync_info = bass_rust.SyncInfo(
                        on_wait=chunks[-1], on_update=list(si.on_update or []))
                out.append(inst)
            bb.instructions = out
    return n_new


def _build(L):
    """Build the Bass module for L GRU steps. Returns nc."""
    assert L >= 1
    nc = bass.Bass("TRN2", target_bir_lowering=False, debug=False)

    statet_d = nc.dram_tensor("statet", [126, 1024], F32R, kind="ExternalInput").ap()
    wa_d = nc.dram_tensor("wa", [8, 126, 1000], F32R, kind="ExternalInput").ap()
    wb_d = nc.dram_tensor("wb", [8, 126, 1000], F32R, kind="ExternalInput").ap()
    hout_d = nc.dram_tensor("hout", [128, HALF], F32R, kind="ExternalOutput").ap()
    sums_d = nc.dram_tensor("sums", [128, 2 * L], F32, kind="ExternalOutput").ap()

    with tile.TileContext(nc) as tc:
        import contextlib
        with contextlib.ExitStack() as ctx:
            consts = ctx.enter_context(tc.tile_pool(name="consts", bufs=1))
            wpool = ctx.enter_context(tc.tile_pool(name="weights", bufs=1))
            hpool = ctx.enter_context(tc.tile_pool(name="hstate", bufs=1))
            work = ctx.enter_context(tc.tile_pool(name="work", bufs=2))
            gpsum = ctx.enter_context(tc.tile_pool(name="gpsum", bufs=2, space="PSUM"))
            tpsum = ctx.enter_context(tc.tile_pool(name="tpsum", bufs=2, space="PSUM"))

            identity = consts.tile([128, 128], F32, tag="identity", name="identity")
            make_identity(nc, identity[:])
            # f32r view for the transposes (verifier wants an f32r producer)
            identity_r = consts.tile([128, 128], F32R, tag="identity_r", name="identity_r")
            nc.vector.tensor_copy(identity_r[:], identity[:])

            statet = wpool.tile([126, 1024], F32R, tag="statet", name="statet")
            nc.gpsimd.dma_start(statet[:], statet_d)
            wa = []
            wb = []
            for u in range(8):
                at = wpool.tile([126, 1000], F32R, tag=f"wa{u}", name=f"wa{u}")
                (nc.gpsimd if u % 2 == 0 else nc.sync).dma_start(at[:], wa_d[u])
                wa.append(at)
            for u in range(8):
                bt = wpool.tile([126, 1000], F32R, tag=f"wb{u}", name=f"wb{u}")
                (nc.gpsimd if u % 2 == 0 else nc.sync).dma_start(bt[:], wb_d[u])
                wb.append(bt)

            hT = [hpool.tile([126, 1024], F32R, tag="hta", name="hta"),
                  hpool.tile([126, 1024], F32R, tag="htb", name="htb")]
            # zero-init (the complementary halves of each K-group must stay 0),
            # then DMA row 125 from the statet image (ones in groups 1 and 3;
            # DVE ops can't start at partition 125, DMA can).
            nc.vector.memzero(hT[0][0:125, :])
            nc.vector.memzero(hT[1][0:125, :])
            nc.gpsimd.dma_start(hT[0][125:126, :], statet_d[125:126, :])
            nc.gpsimd.dma_start(hT[1][125:126, :], statet_d[125:126, :])

            sums = consts.tile([128, 2 * L], F32, tag="sums", name="sums")

            hprev = None  # packed [128, 250] h of the previous step
            hnew = None
            for k in range(1, L + 1):
                first = k == 1
                lhs_tile = statet if first else hT[k % 2]
                W = wa if first else wb

                # separate PSUM tiles per bank so the rz consumers don't
                # wait on the n-block matmuls
                grz = gpsum.tile([128, 512], F32, tag="grz", name="grz")
                gn = gpsum.tile([128, 512], F32, tag="gn", name="gn")
                def mm_half(out_ap, c0, us):
                    # groups 0..3 depend only on the pA copies; 4..7 on pB
                    for u in us:
                        ku = 126 if u in (1, 3) else 125
                        lhsT = lhs_tile[0:ku, 128 * u:128 * u + 128]
                        nc.tensor.matmul(out_ap, lhsT,
                                         W[u][0:ku, c0:c0 + 500],
                                         start=(u == 0), stop=(u == 7))

                # region 1 = [gr | gz]: both sigmoids + the whole z-path
                # run while region 2 ([gin | ghn]) is still streaming
                mm_half(grz[:, 0:500], 0, range(8))

                rz = work.tile([128, 2 * HALF], F32, tag="rz", name="rz")
                nc.scalar.activation(rz[:], grz[:, 0:500], AF.Sigmoid)
                r = rz[:, 0:250]
                z = rz[:, 250:500]
                zm1 = work.tile([128, HALF], F32, tag="zm1", name="zm1")
                nc.vector.tensor_scalar_sub(zm1[:], z, 1.0)
                zh = work.tile([128, HALF], F32, tag="zh", name="zh")
                if not first:
                    nc.vector.tensor_mul(zh[:], z, hprev[:])

                mm_half(gn[:, 0:500], 500, range(8))

                # chunked chain (2 x 125 cols) so transpose A can start while
                # chunk 1 is still in the vector/scalar pipes
                rhn = work.tile([128, HALF], F32R, tag="rhn", name="rhn")
                targ = work.tile([128, HALF], F32R, tag="targ", name="targ")
                n = work.tile([128, HALF], F32R, tag="n", name="n")
                t2 = work.tile([128, HALF], F32R, tag="t2", name="t2")
                hnew = work.tile([128, HALF], F32R, tag="hnew", name="hnew")
                if k < L:
                    dst = hT[(k + 1) % 2]
                    dstv = dst[:].rearrange("p (u c) -> p u c", c=128)
                for c in (0, 1):
                    s = slice(125 * c, 125 * (c + 1))
                    acc = sums[:, 2 * (k - 1) + c:2 * (k - 1) + c + 1]
                    nc.vector.tensor_mul(rhn[:, s], r[:, s], gn[:, 250 + 125 * c:250 + 125 * (c + 1)])
                    nc.vector.tensor_add(targ[:, s], rhn[:, s], gn[:, 125 * c:125 * (c + 1)])
                    nc.scalar.activation(n[:, s], targ[:, s], AF.Tanh)
                    if first:
                        # h == 0: h_new = n * (1 - z) = (-n) * (z - 1)
                        nc.vector.scalar_tensor_tensor(
                            hnew[:, s], n[:, s], -1.0, zm1[:, s],
                            op0=ALU.mult, op1=ALU.mult, accum_out=acc)
                    else:
                        nc.vector.scalar_tensor_tensor(
                            t2[:, s], n[:, s], -1.0, zm1[:, s],
                            op0=ALU.mult, op1=ALU.mult)
                        nc.vector.scalar_tensor_tensor(
                            hnew[:, s], t2[:, s], 0.0, zh[:, s],
                            op0=ALU.bypass, op1=ALU.add, accum_out=acc)
                    if k < L:
                        if c == 0:
                            # tiny write-only matmuls on mid-chain tensors:
                            # real PE activity spaced through the tail so the
                            # HAM idle window never completes a full period
                            dmy = tpsum.tile([1, 128], F32, tag="dmy",
                                             name="dmy", bufs=1)
                            for anchor in (rhn, targ, n, t2 if not first else n):
                                nc.tensor.matmul(dmy[:], anchor[0:128, 0:1],
                                                 identity_r[:, 0:128])
                            pA = tpsum.tile([125, 128], F32, tag="pT", name="pA")
                            # transpose via a REGULAR matmul (in.T @ I): unlike
                            # transpose-mode this counts as PE activity for the
                            # HAM clock gate, keeping the array at full clock
                            # through the serial gate-chain tail
                            nc.tensor.matmul(pA[:], hnew[:, 0:125],
                                             identity_r[:])
                            pAv = pA[:].rearrange("p (u c) -> p u c", c=64)
                            # groups 0,1 (G0) on ACT and 2,3 (G1) on DVE run
                            # in parallel -> first 4 next-step matmuls unblock
                            nc.scalar.copy(dstv[0:125, 0:2, 0:64], pAv)
                            nc.vector.tensor_copy(dstv[0:125, 2:4, 64:128], pAv)
                        else:
                            pB = tpsum.tile([125, 128], F32, tag="pT", name="pB")
                            nc.tensor.matmul(pB[:], hnew[:, 125:250],
                                             identity_r[:])
                            pBv = pB[:].rearrange("p (u c) -> p u c", c=64)
                            nc.scalar.copy(dstv[0:125, 4:6, 0:64], pBv)
                            nc.vector.tensor_copy(dstv[0:125, 6:8, 64:128], pBv)
                hprev = hnew

            nc.gpsimd.dma_start(hout_d, hnew[:])
            nc.gpsimd.dma_start(sums_d, sums[:])

    _split_overwide_waits(nc)
    return nc


_NC_CACHE = {}


def _get_nc(L):
    if L not in _NC_CACHE:
        _NC_CACHE[L] = _build(L)
    return _NC_CACHE[L]


def _prep_weights(W_ih, W_hh, b_ih, b_hh):
    """Build wa/wb DRAM images [8, 126, 1000] (grouped, permuted, bias rows)."""
    W_ih = np.asarray(W_ih, np.float32)
    W_hh = np.asarray(W_hh, np.float32)
    b_ih = np.asarray(b_ih, np.float32)
    b_hh = np.asarray(b_hh, np.float32)

    def full(rz_src, n_rows, bias_rz, bias_n):
        rz = rz_src[np.ix_(GATE_ROWS_RZ, PERM)].T          # [500, 1000]
        nn_ = n_rows[:, PERM].T                            # [500, 1000]
        top = np.hstack([rz, nn_])                         # [500, 2000]
        bias = np.hstack([bias_rz, bias_n])[None, :]       # [1, 2000]
        return np.vstack([top, bias]).astype(np.float32)   # [501, 2000]

    zeros = np.zeros((250, D), np.float32)
    bias_rz_sum = (b_ih[:1000] + b_hh[:1000])[GATE_ROWS_RZ]
    bias_n = np.concatenate([b_ih[1000:1250], b_hh[1000:1250],
                             b_ih[1250:1500], b_hh[1250:1500]])

    WB = full(W_ih[:1000] + W_hh[:1000],
              np.vstack([W_ih[1000:1250], W_hh[1000:1250],
                         W_ih[1250:1500], W_hh[1250:1500]]),
              bias_rz_sum, bias_n)
    WA = full(W_ih[:1000],
              np.vstack([W_ih[1000:1250], zeros,
                         W_ih[1250:1500], zeros]),
              bias_rz_sum, bias_n)

    # group u -> (D-block index into the PERM layout, gate-half G)
    DBLK = (0, 1, 0, 1, 2, 3, 2, 3)
    GHALF = (0, 0, 1, 1, 0, 0, 1, 1)

    def pack(Wf):
        out = np.zeros((8, 126, 1000), np.float32)
        for u in range(8):
            t = DBLK[u]
            lo, hi = (0, 1000) if GHALF[u] == 0 else (500, 1500)
            # region1 = [r | z], region2 = [in | hn]
            rows = Wf[125 * t:125 * (t + 1)]
            out[u, 0:125, 0:500] = rows[:, lo:lo + 500]
            out[u, 0:125, 500:1000] = rows[:, hi:hi + 500]
        # bias row: once per gate-half (groups 1 and 3 have the ones row)
        for u, (lo, hi) in ((1, (0, 1000)), (3, (500, 1500))):
            out[u, 125, 0:500] = Wf[500, lo:lo + 500]
            out[u, 125, 500:1000] = Wf[500, hi:hi + 500]
        return out

    return pack(WA), pack(WB)


def _prep_state(state):
    """Per-core stationary state^T images [126, 1024]."""
    state = np.asarray(state, np.float32)
    outs = []
    for c in range(NCORES):
        shard = state[BS * c:BS * (c + 1)]            # [64, 500]
        st = shard[:, PERM].T                         # [500, 64]
        img = np.zeros((126, 1024), np.float32)
        DBLK = (0, 1, 0, 1, 2, 3, 2, 3)
        GHALF = (0, 0, 1, 1, 0, 0, 1, 1)
        for u in range(8):
            rows = st[125 * DBLK[u]:125 * (DBLK[u] + 1)]
            off = 128 * u + 64 * GHALF[u]
            img[0:125, off:off + 64] = rows
        img[125, 128 * 1:128 * 1 + 64] = 1.0
        img[125, 128 * 3 + 64:128 * 3 + 128] = 1.0
        outs.append(img)
    return outs


def _run(L, stateTs, wa, wb, trace=False):
    nc = _get_nc(L)
    in_maps = [{"statet": np.ascontiguousarray(stateTs[c]),
                "wa": wa, "wb": wb} for c in range(NCORES)]
    res = bass_utils.run_bass_kernel_spmd(
        nc, in_maps, core_ids=list(range(NCORES)), trace=trace)
    shards = []
    sums = np.zeros((128, 2 * L), np.float64)
    for c in range(NCORES):
        hout = res.results[c]["hout"]
        shards.append(np.concatenate([hout[0:64], hout[64:128]], axis=1))
        sums += res.results[c]["sums"].astype(np.float64)
    h = np.concatenate(shards, axis=0)                # [512, 500]
    means = (sums[:, 0::2] + sums[:, 1::2]).sum(axis=0) / (B * D)  # [L]
    return h, means, res


def kernel(state, W_ih, W_hh, b_ih, b_hh, break_condition, recursion_limit):
    state = np.asarray(state, np.float32)
    L = int(np.asarray(recursion_limit))
    if L <= 0:
        return state.copy()
    bc = float(np.asarray(break_condition))

    wa, wb = _prep_weights(W_ih, W_hh, b_ih, b_hh)
    stateTs = _prep_state(state)

    h, means, _ = _run(L, stateTs, wa, wb)
    fired = np.nonzero(means > bc)[0]
    if fired.size and fired[0] + 1 < L:
        # break fired at step k* = fired[0]+1: output latches h_{k*}
        h, _, _ = _run(int(fired[0]) + 1, stateTs, wa, wb)
    return h.astype(np.float32)
